# Initial kernel scaffold
#
"""MixtureLinearAttention TRN2 kernel (8 NeuronCores, SPMD).

Math (per batch n, component c, head h):
  Qf = elu(q @ W_c) + 1 ;  Kf = (elu(k @ W_c) + 1) * mask
  KVt[e, m] = sum_s Kf[s,e] V[s,m] ;  Ksum[e] = sum_s Kf[s,e]
  Den[s] = sum_e Qf[s,e] Ksum[e]
  out[s,h,m] = sum_c softmax(mix)_c / Den[s] * sum_e Qf[s,e] KVt[e,m]

Sharding: core i -> (n = i//2, heads hh = (i%2)*8..+8). Host does all layout
transposes (q/k fed d-major; output returned m-major and transposed back).
Device pipeline per h-pair (hp) in transposed feature space, float32r matmuls:

  phi-K:  lhsT = kT-pair chunk, rhs = blockdiag(Wcat)   -> Kf[s, (h2,c,e)]
          ACT Exp + DVE min/relu-add finish (elu(x)+1 = min(e^x,1)+relu(x))
  KV:     lhsT = Vaug(s,65), rhs = Kf slices -> [65, (c,e)] psum accumulate
  KVt:    transpose via identity matmuls -> [64e, 65] per (h,c) (col64=Ksum)
  kvstack: SBUF->SBUF DMA builds c-pair stacks [(ce|c'e), 65] per (h, p)
  phi-Q:  lhsT = diag(W_c,W_c'), rhs = duplicated qT(h) -> Qf^T c-pair stacks
  Den:    4 accumulating matmuls -> [128, s] psum with 8 live rows (h2, c)
  1/x:    ACT Ln then Exp(scale=-1)  (same table set -> no reloads)
  Zrep:   selector matmuls replicate z rows -> [(ce|c'e), s]
  zQf:    DVE multiply ;  OutT: lhsT = kvstack, rhs = zQf, accumulate over p
"""
import sys

if "/opt/trn_rl_repo" not in sys.path:
    sys.path.insert(0, "/opt/trn_rl_repo")

from contextlib import ExitStack

import numpy as np

import concourse.bass as bass
import concourse.tile as tile
from concourse import mybir
from concourse.masks import make_identity

F32 = mybir.dt.float32
F32R = mybir.dt.float32r
ALU = mybir.AluOpType
AFT = mybir.ActivationFunctionType

N, S, H, D, C = 4, 2048, 16, 64, 4
E = M = 64
HL = 8          # heads per core
NHP = HL // 2   # h-pairs
NCHUNK = S // 128


def _split_multiwait(nc, max_waits=1):
    """This walrus build rejects >1 sync wait per instruction; hoist extra
    waits onto NoOps inserted just before, on the same engine."""
    k = 0
    for fn in nc.m.functions:
        for bb in fn.blocks:
            out, changed = [], False
            for inst in bb.instructions:
                si = inst.sync_info
                if si is not None and si.on_wait and len(si.on_wait) > max_waits:
                    waits = list(si.on_wait)
                    while len(waits) > max_waits:
                        chunk, waits = waits[:max_waits], waits[max_waits:]
                        nop = mybir.InstNoOp(name=f"wait_split_{k}", ins=[], outs=[])
                        k += 1
                        nop.engine = inst.engine
                        nop.sync_info = mybir.SyncInfo(on_wait=chunk, on_update=[])
                        out.append(nop)
                        changed = True
                    inst.sync_info = mybir.SyncInfo(
                        on_wait=waits, on_update=list(si.on_update or [])
                    )
                out.append(inst)
            if changed:
                bb.instructions = out


def build_program():
    nc = bass.Bass("TRN2", debug=False)
    ap = {}
    ap["qTd"] = nc.dram_tensor("qTd", [HL, 128, S], F32R, kind="ExternalInput").ap()
    ap["kT"] = nc.dram_tensor("kT", [NHP, 128, S], F32R, kind="ExternalInput").ap()
    ap["vaug"] = nc.dram_tensor("vaug", [HL, 128, NCHUNK * 65], F32R, kind="ExternalInput").ap()
    ap["wq2"] = nc.dram_tensor("wq2", [2, 128, 128], F32R, kind="ExternalInput").ap()
    ap["wk"] = nc.dram_tensor("wk", [128, 512], F32R, kind="ExternalInput").ap()
    ap["wmask"] = nc.dram_tensor("wmask", [128, 6], F32, kind="ExternalInput").ap()
    ap["sel2"] = nc.dram_tensor("sel2", [4, 128, 128], F32R, kind="ExternalInput").ap()
    ap["outT"] = nc.dram_tensor("outT", [HL, 64, S], F32, kind="ExternalOutput").ap()

    tc = tile.TileContext(nc)
    with tc:
        with ExitStack() as ctx:
            cpool = ctx.enter_context(tc.tile_pool(name="consts", bufs=1))
            wq_t = []
            for p in range(2):
                w1 = cpool.tile([128, 128], F32R, name=f"wq{p}", tag=f"wq{p}")
                nc.sync.dma_start(w1[:], ap["wq2"][p])
                wq_t.append(w1)
            wk_t = cpool.tile([128, 512], F32R)
            nc.sync.dma_start(wk_t[:], ap["wk"][:])
            wmask_t = cpool.tile([128, 6], F32)
            nc.sync.dma_start(wmask_t[:], ap["wmask"][:])
            sel_t = []
            for i in range(4):
                s1 = cpool.tile([128, 128], F32R, name=f"sel{i}", tag=f"sel{i}")
                nc.sync.dma_start(s1[:], ap["sel2"][i])
                sel_t.append(s1)
            ident = cpool.tile([128, 128], F32)
            make_identity(nc, ident[:])

            qd_pool = ctx.enter_context(tc.tile_pool(name="qd", bufs=1))
            kt_pool = ctx.enter_context(tc.tile_pool(name="kt", bufs=2))
            v_pool = ctx.enter_context(tc.tile_pool(name="v", bufs=2))
            kf_pool = ctx.enter_context(tc.tile_pool(name="kf", bufs=8))
            etK_pool = ctx.enter_context(tc.tile_pool(name="etK", bufs=2))
            etQ_pool = ctx.enter_context(tc.tile_pool(name="etQ", bufs=2))
            qf_pool = ctx.enter_context(tc.tile_pool(name="qf", bufs=5))
            kvs_pool = ctx.enter_context(tc.tile_pool(name="kvs", bufs=2))
            kstk_pool = ctx.enter_context(tc.tile_pool(name="kstk", bufs=2))
            dl_pool = ctx.enter_context(tc.tile_pool(name="dl", bufs=2))
            z_pool = ctx.enter_context(tc.tile_pool(name="z", bufs=2))
            lnt_pool = ctx.enter_context(tc.tile_pool(name="lnt", bufs=2))
            zq_pool = ctx.enter_context(tc.tile_pool(name="zq", bufs=2))
            ob_pool = ctx.enter_context(tc.tile_pool(name="ob", bufs=3))
            # PSUM (8 banks): bigK 2 + bigQ 2 + kv 1 + sm 2 + out 1
            ps_bigK = ctx.enter_context(tc.tile_pool(name="psbigK", bufs=1, space="PSUM"))
            ps_bigQ = ctx.enter_context(tc.tile_pool(name="psbigQ", bufs=1, space="PSUM"))
            ps_kv = ctx.enter_context(tc.tile_pool(name="pskv", bufs=1, space="PSUM"))
            ps_sm = ctx.enter_context(tc.tile_pool(name="pssm", bufs=2, space="PSUM"))
            ps_out = ctx.enter_context(tc.tile_pool(name="psout", bufs=1, space="PSUM"))

            for hp in range(NHP):
                h0 = 2 * hp
                kt_t = kt_pool.tile([128, S], F32R, name="kt")
                nc.sync.dma_start(kt_t[:], ap["kT"][hp])
                v_ts = []
                for j in range(2):
                    v_t = v_pool.tile([128, NCHUNK * 65], F32R, name=f"v{j}", tag=f"v{j}")
                    nc.sync.dma_start(v_t[:], ap["vaug"][h0 + j])
                    v_ts.append(v_t)
                qd_ts = []
                for j in range(2):
                    qd_t = qd_pool.tile([128, S], F32R, name=f"qd{j}", tag=f"qd{j}")
                    nc.sync.dma_start(qd_t[:], ap["qTd"][h0 + j])
                    qd_ts.append(qd_t)

                # ---- phi-K + elu finish + KV accumulate ----
                kv_ps = ps_kv.tile([65, 512], F32, name="kvps", tag="kvps")
                for g in range(8):
                    kf_ps = ps_bigK.tile([128, 1024], F32, name="kfps", tag="bigK")
                    for cc in range(2):
                        ch = 2 * g + cc
                        nc.tensor.matmul(
                            kf_ps[:, 512 * cc : 512 * cc + 512],
                            kt_t[:, 128 * ch : 128 * ch + 128],
                            wk_t[:],
                            start=True,
                            stop=True,
                        )
                    ek = etK_pool.tile([128, 1024], F32, name="ek", tag="ek")
                    nc.scalar.activation(ek[:], kf_ps[:], AFT.Exp)
                    akk = etK_pool.tile([128, 1024], F32, name="akk", tag="akk")
                    nc.vector.tensor_scalar(akk[:], ek[:], 1.0, None, ALU.min)
                    kf = kf_pool.tile([128, 1024], F32R, name="kf")
                    nc.vector.scalar_tensor_tensor(
                        kf[:], kf_ps[:], 0.0, akk[:], ALU.max, ALU.add
                    )
                    for j in range(2):
                        for cc in range(2):
                            ch = 2 * g + cc
                            nc.tensor.matmul(
                                kv_ps[:, 256 * j : 256 * j + 256],
                                v_ts[j][:, 65 * ch : 65 * ch + 65],
                                kf[:, 512 * cc + 256 * j : 512 * cc + 256 * j + 256],
                                start=(ch == 0 and j == 0),
                                stop=(ch == NCHUNK - 1),
                            )

                # ---- KVt: transposes via identity matmuls, then c-pair stacks ----
                kvs = []   # per h2: [64, 260] f32r ([e, (c, m'+ksum)])
                for j in range(2):
                    kvc_t = kvs_pool.tile([65, 256], F32, name=f"kvc{j}", tag=f"kvc{j}")
                    nc.vector.tensor_copy(kvc_t[:], kv_ps[:, 256 * j : 256 * j + 256])
                    tp_ps = ps_sm.tile([64, 260], F32, name="tpps", tag="sm")
                    for c in range(C):
                        nc.tensor.matmul(
                            tp_ps[:, 65 * c : 65 * c + 65],
                            kvc_t[:, 64 * c : 64 * c + 64],
                            ident[:65, :65],
                            start=(c == 0),
                            stop=True,
                        )
                    kvs_t = kvs_pool.tile([64, 260], F32R, name=f"kvs{j}", tag=f"kvs{j}")
                    nc.vector.tensor_copy(kvs_t[:], tp_ps[:])
                    kvs.append(kvs_t)
                # Wait: transpose of kvc slice [65, 64] gives [64, 65]; but the
                # matmul above computes kvc_slice.T @ ident[:65,:65] with
                # contraction 65 -> out [64, 65]. Free dim of out is 65, so the
                # psum slice must be 65 wide. (tp_ps sliced accordingly.)
                kstk = {}
                for j in range(2):
                    for p in range(2):
                        kst = kstk_pool.tile(
                            [128, 65], F32R, name=f"kstk{j}{p}", tag=f"kstk{j}{p}"
                        )
                        nc.sync.dma_start(
                            kst[0:64, :], kvs[j][:, 65 * (2 * p) : 65 * (2 * p) + 65]
                        )
                        nc.sync.dma_start(
                            kst[64:128, :],
                            kvs[j][:, 65 * (2 * p + 1) : 65 * (2 * p + 1) + 65],
                        )
                        kstk[(j, p)] = kst

                # ---- phi-Q (c-pair stacked) + elu finish ----
                qf2 = {}
                for j in range(2):
                    for p in range(2):
                        qf_t = qf_pool.tile([128, S], F32R, name=f"qf{j}{p}", tag="qf")
                        for half in range(2):
                            pq_ps = ps_bigQ.tile([128, 1024], F32, name="pqps", tag="bigQ")
                            for g in range(2):
                                o = 1024 * half + 512 * g
                                nc.tensor.matmul(
                                    pq_ps[:, 512 * g : 512 * g + 512],
                                    wq_t[p][:],
                                    qd_ts[j][:, o : o + 512],
                                    start=True,
                                    stop=True,
                                )
                            eq = etQ_pool.tile([128, 1024], F32, name="eq", tag="eq")
                            nc.scalar.activation(eq[:], pq_ps[:], AFT.Exp)
                            aq = etQ_pool.tile([128, 1024], F32, name="aq", tag="aq")
                            nc.vector.tensor_scalar(aq[:], eq[:], 1.0, None, ALU.min)
                            nc.vector.scalar_tensor_tensor(
                                qf_t[:, 1024 * half : 1024 * half + 1024],
                                pq_ps[:],
                                0.0,
                                aq[:],
                                ALU.max,
                                ALU.add,
                            )
                        qf2[(j, p)] = qf_t

                # ---- Den lhsT [128, 128] per (j, p) ----
                dlw = {}
                for j in range(2):
                    for p in range(2):
                        dl = dl_pool.tile([128, 128], F32R, name=f"dl{j}{p}", tag=f"dl{j}{p}")
                        nc.vector.tensor_scalar(dl[:], wq_t[0][:], 0.0, None, ALU.mult)
                        base = 4 * j + 2 * p
                        ks = kstk[(j, p)][:, 64:65]
                        nc.vector.tensor_scalar(
                            dl[:, base : base + 1], ks, wmask_t[:, 3 * p : 3 * p + 1],
                            None, ALU.mult,
                        )
                        nc.vector.tensor_scalar(
                            dl[:, base + 1 : base + 2], ks,
                            wmask_t[:, 3 * p + 1 : 3 * p + 2], None, ALU.mult,
                        )
                        if j == 0 and p == 0:
                            nc.vector.tensor_scalar(
                                dl[:, 8:128], ks.to_broadcast((128, 120)),
                                wmask_t[:, 2:3], None, ALU.mult,
                            )
                        dlw[(j, p)] = dl

                # ---- Den + z + Zrep + zQf + OutT per 512-chunk ----
                for ch4 in range(4):
                    sl = slice(512 * ch4, 512 * ch4 + 512)
                    den_ps = ps_sm.tile([128, 512], F32, name="denps", tag="sm")
                    for i, (j, p) in enumerate(((0, 0), (0, 1), (1, 0), (1, 1))):
                        nc.tensor.matmul(
                            den_ps[:],
                            dlw[(j, p)][:],
                            qf2[(j, p)][:, sl],
                            start=(i == 0),
                            stop=(i == 3),
                        )
                    lnt = lnt_pool.tile([128, 512], F32, name="lnt", tag="lnt")
                    nc.scalar.activation(lnt[:], den_ps[:], AFT.Ln)
                    z_t = z_pool.tile([128, 512], F32R, name="zt", tag="z")
                    nc.scalar.activation(z_t[:], lnt[:], AFT.Exp, scale=-1.0)

                    ot_ps = {}
                    for j in range(2):
                        ot_ps[j] = ps_out.tile([64, 512], F32, name=f"otps{j}", tag="ot")
                    for j in range(2):
                        for p in range(2):
                            zrep_ps = ps_sm.tile([128, 512], F32, name="zrepps", tag="sm")
                            nc.tensor.matmul(
                                zrep_ps[:], sel_t[2 * j + p][:], z_t[:],
                                start=True, stop=True,
                            )
                            zq_t = zq_pool.tile([128, 512], F32R, name="zqt", tag="zq")
                            nc.vector.tensor_tensor(
                                zq_t[:], qf2[(j, p)][:, sl], zrep_ps[:], ALU.mult
                            )
                            nc.tensor.matmul(
                                ot_ps[j][:],
                                kstk[(j, p)][:, 0:64],
                                zq_t[:],
                                start=(p == 0),
                                stop=(p == 1),
                            )
                    for j in range(2):
                        ob5 = ob_pool.tile([64, 512], F32, name="ob5", tag="ob")
                        nc.vector.tensor_copy(ob5[:], ot_ps[j][:])
                        nc.sync.dma_start(ap["outT"][h0 + j][:, sl], ob5[:])

    _split_multiwait(nc)
    return nc


_NC_CACHE = None


def _get_nc():
    global _NC_CACHE
    if _NC_CACHE is None:
        _NC_CACHE = build_program()
    return _NC_CACHE


def _softmax(x):
    e = np.exp(x - x.max())
    return e / e.sum()


def prep_core_inputs(queries, keys, values, key_mask, feat_W, mix_weights, core):
    n, hh = core // 2, (core % 2) * HL
    W = _softmax(np.asarray(mix_weights, np.float64)).astype(np.float32)

    qs = queries[n][:, hh : hh + HL, :].transpose(1, 2, 0)  # [HL, D, S]
    qTd = np.ascontiguousarray(
        np.concatenate([qs, qs], axis=1)
    ).astype(np.float32)  # [HL, 128, S]
    ks = keys[n][:, hh : hh + HL, :].transpose(1, 2, 0)
    kT = np.ascontiguousarray(ks.reshape(NHP, 128, S)).astype(np.float32)

    mask = key_mask[n].astype(np.float32)
    vm = values[n][:, hh : hh + HL, :] * mask[:, None, None]
    vaug = np.concatenate(
        [vm, np.broadcast_to(mask[:, None, None], (S, HL, 1))], axis=2
    )
    vaug = vaug.transpose(1, 0, 2).reshape(HL, NCHUNK, 128, 65)
    vaug = np.ascontiguousarray(vaug.transpose(0, 2, 1, 3)).reshape(
        HL, 128, NCHUNK * 65
    ).astype(np.float32)

    wq2 = np.zeros((2, 128, 128), np.float32)
    for p in range(2):
        wq2[p, :64, :64] = feat_W[2 * p]
        wq2[p, 64:, 64:] = feat_W[2 * p + 1]
    wcat = np.concatenate([feat_W[c] for c in range(C)], axis=1)
    wk = np.zeros((128, 512), np.float32)
    wk[:64, :256] = wcat
    wk[64:, 256:] = wcat

    wmask = np.zeros((128, 6), np.float32)
    for p in range(2):
        wmask[:64, 3 * p + 0] = 1.0 / W[2 * p]
        wmask[64:, 3 * p + 1] = 1.0 / W[2 * p + 1]
        wmask[:64, 3 * p + 2] = 1.0 / W[2 * p]
        wmask[64:, 3 * p + 2] = 1.0 / W[2 * p + 1]

    sel2 = np.zeros((4, 128, 128), np.float32)
    for j in range(2):
        for p in range(2):
            base = 4 * j + 2 * p
            sel2[2 * j + p, base, :64] = 1.0
            sel2[2 * j + p, base + 1, 64:] = 1.0

    return {"qTd": qTd, "kT": kT, "vaug": vaug, "wq2": wq2, "wk": wk,
            "wmask": wmask, "sel2": sel2}


def run_cores(inputs, trace=False, tmpdir=None):
    from concourse.bass_utils import run_bass_kernel_spmd

    nc = _get_nc()
    in_maps = [prep_core_inputs(**inputs, core=i) for i in range(8)]
    kwargs = {}
    if trace:
        kwargs = {"trace": True, "tmpdir": tmpdir}
    res = run_bass_kernel_spmd(nc, in_maps, core_ids=list(range(8)), **kwargs)
    out = np.empty((N, S, H, M), np.float32)
    for i in range(8):
        n, hh = i // 2, (i % 2) * HL
        oT = res.results[i]["outT"]  # [HL, 64, S]
        for h in range(HL):
            out[n, :, hh + h, :] = oT[h].T
    return out, res


def kernel(queries, keys, values, key_mask, feat_W, mix_weights):
    out, _ = run_cores(
        dict(queries=np.asarray(queries), keys=np.asarray(keys),
             values=np.asarray(values), key_mask=np.asarray(key_mask),
             feat_W=np.asarray(feat_W), mix_weights=np.asarray(mix_weights))
    )
    return out



# revision 41
# speedup vs baseline: 1.3666x; 1.3666x over previous
"""MixtureLinearAttention TRN2 kernel (8 NeuronCores, SPMD).

Math (per batch n, component c, head h):
  Qf = elu(q @ W_c) + 1 ;  Kf = (elu(k @ W_c) + 1) * mask
  KVt[e, m] = sum_s Kf[s,e] V[s,m] ;  Ksum[e] = sum_s Kf[s,e]
  Den[s] = sum_e Qf[s,e] Ksum[e]
  out[s,h,m] = sum_c softmax(mix)_c / Den[s] * sum_e Qf[s,e] KVt[e,m]

Sharding: core i -> (n = i//2, heads hh = (i%2)*8..+8). Host does all layout
transposes. All matmul operands fp16 (fp32 HIGH-mode matmuls are 4x slower);
PSUM accumulation stays fp32 so only input-rounding error is added.

Per h-pair pipeline:
  phi-K:  lhsT = kT chunk, rhs = blockdiag(Wcat)  -> proj psum [128s, 1024]
          ACT Exp -> ek fp16 ; DVE ts min(ek,1) -> akk
          DVE stt (proj max 0) add akk -> kf fp16
  KV:     lhsT = Vaug(s,65) fp16, rhs = kf slices -> [65, 512] psum f32
  kvc:    DVE ts (kv * 1/256) -> fp16  (256-scale keeps fp16 zq normal)
  KVt:    transposes via fp16 identity matmuls; c-pair stacks built directly
          in one [128, 260] psum tile via tile_position col offset 0/64.
          NOTE: psum start_tensor_calc resets per-PARTITION: the first MM
          writing each partition range of a bank needs start=True.
  phi-Q:  lhsT = diag(W_c,W_c') fp16 -> qf fp16 c-pair stacks [128ce, S]
          ACT Exp + ACT Relu (psum drains) ; DVE min + DVE add (sbuf fp16)
  Den:    per-stack lhsT [128,8] (ksum/256 * 1/W cols) -> den [8, 512] psum
          (shares the kv psum ring; sharing with zrep/out rings stalls PE)
  1/x:    ACT Ln -> ACT Exp(scale=-1) -> z fp16 [8, 2048]   (z = 256W/den)
  zrep:   selector matmuls [8,128]x[8,512] -> [128, 512] psum per stack
          (DMA/DVE cannot partition-broadcast; gpsimd can but is too slow)
  zq:     DVE tensor_tensor(qf, zrep_psum) ; OutT: packed [128, 512] psum,
          both heads via tile_position, accumulate over p
  out:    ACT-Copy / DVE-ts alternate psum -> sbuf fp16, DMA to DRAM
          (DMA cannot read PSUM; gpsimd cannot access PSUM at all)
"""
import sys

if "/opt/trn_rl_repo" not in sys.path:
    sys.path.insert(0, "/opt/trn_rl_repo")

from contextlib import ExitStack

import numpy as np

import concourse.bass as bass
import concourse.tile as tile
from concourse import mybir
from concourse.masks import make_identity

F32 = mybir.dt.float32
F16 = mybir.dt.float16
ALU = mybir.AluOpType
AFT = mybir.ActivationFunctionType

N, S, H, D, C = 4, 2048, 16, 64, 4
E = M = 64
HL = 8          # heads per core
NHP = HL // 2   # h-pairs
NCHUNK = S // 128
OSCALE = 256.0  # kv scaled by 1/256, z by 256 -> fp16 zq stays normal


def _split_multiwait(nc, max_waits=1):
    """This walrus build rejects >1 sync wait per instruction; hoist extra
    waits onto NoOps inserted just before, on the same engine."""
    k = 0
    for fn in nc.m.functions:
        for bb in fn.blocks:
            out, changed = [], False
            for inst in bb.instructions:
                si = inst.sync_info
                if si is not None and si.on_wait and len(si.on_wait) > max_waits:
                    waits = list(si.on_wait)
                    while len(waits) > max_waits:
                        chunk, waits = waits[:max_waits], waits[max_waits:]
                        nop = mybir.InstNoOp(name=f"wait_split_{k}", ins=[], outs=[])
                        k += 1
                        nop.engine = inst.engine
                        nop.sync_info = mybir.SyncInfo(on_wait=chunk, on_update=[])
                        out.append(nop)
                        changed = True
                    inst.sync_info = mybir.SyncInfo(
                        on_wait=waits, on_update=list(si.on_update or [])
                    )
                out.append(inst)
            if changed:
                bb.instructions = out


def build_program():
    nc = bass.Bass("TRN2", debug=False)
    ap = {}
    ap["qTd"] = nc.dram_tensor("qTd", [HL, 128, S], F16, kind="ExternalInput").ap()
    ap["kT"] = nc.dram_tensor("kT", [NHP, 128, S], F16, kind="ExternalInput").ap()
    ap["vaug"] = nc.dram_tensor("vaug", [HL, 128, NCHUNK * 65], F16, kind="ExternalInput").ap()
    ap["wq2"] = nc.dram_tensor("wq2", [2, 128, 128], F16, kind="ExternalInput").ap()
    ap["wk"] = nc.dram_tensor("wk", [128, 512], F16, kind="ExternalInput").ap()
    ap["wmask"] = nc.dram_tensor("wmask", [128, 6], F32, kind="ExternalInput").ap()
    ap["sel"] = nc.dram_tensor("sel", [8, 512], F16, kind="ExternalInput").ap()
    ap["outT"] = nc.dram_tensor("outT", [HL * 64, S], F16, kind="ExternalOutput").ap()

    tc = tile.TileContext(nc)
    with tc:
        with ExitStack() as ctx:
            cpool = ctx.enter_context(tc.tile_pool(name="consts", bufs=1))
            wq_t = []
            for p in range(2):
                w1 = cpool.tile([128, 128], F16, name=f"wq{p}", tag=f"wq{p}")
                nc.sync.dma_start(w1[:], ap["wq2"][p])
                wq_t.append(w1)
            wk_t = cpool.tile([128, 512], F16)
            nc.sync.dma_start(wk_t[:], ap["wk"][:])
            wmask_t = cpool.tile([128, 6], F32)
            nc.sync.dma_start(wmask_t[:], ap["wmask"][:])
            ident = cpool.tile([128, 128], F16)
            make_identity(nc, ident[:])
            sel_t = cpool.tile([8, 512], F16, name="sel", tag="sel")
            nc.sync.dma_start(sel_t[:], ap["sel"][:])

            qd_pool = ctx.enter_context(tc.tile_pool(name="qd", bufs=3))
            kt_pool = ctx.enter_context(tc.tile_pool(name="kt", bufs=3))
            v_pool = ctx.enter_context(tc.tile_pool(name="v", bufs=3))
            kf_pool = ctx.enter_context(tc.tile_pool(name="kf", bufs=10))
            etK_pool = ctx.enter_context(tc.tile_pool(name="etK", bufs=4))
            etQ_pool = ctx.enter_context(tc.tile_pool(name="etQ", bufs=4))
            qf_pool = ctx.enter_context(tc.tile_pool(name="qf", bufs=8))
            kvc_pool = ctx.enter_context(tc.tile_pool(name="kvc", bufs=3))
            kstk_pool = ctx.enter_context(tc.tile_pool(name="kstk", bufs=3))
            dl_pool = ctx.enter_context(tc.tile_pool(name="dl", bufs=2))
            z_pool = ctx.enter_context(tc.tile_pool(name="z", bufs=3))
            lnt_pool = ctx.enter_context(tc.tile_pool(name="lnt", bufs=3))
            zq_pool = ctx.enter_context(tc.tile_pool(name="zq", bufs=4))
            ob_pool = ctx.enter_context(tc.tile_pool(name="ob", bufs=4))
            # PSUM (8 banks): bigK 2 + bigQ 2 + kv 1 + [tpk|den|zrep] 2 + out 1
            ps_bigK = ctx.enter_context(tc.tile_pool(name="psbigK", bufs=2, space="PSUM"))
            ps_bigQ = ctx.enter_context(tc.tile_pool(name="psbigQ", bufs=1, space="PSUM"))
            ps_kv = ctx.enter_context(tc.tile_pool(name="pskv", bufs=1, space="PSUM"))
            ps_tpk = ctx.enter_context(tc.tile_pool(name="pstpk", bufs=2, space="PSUM"))
            ps_out = ctx.enter_context(tc.tile_pool(name="psout", bufs=1, space="PSUM"))

            for hp in range(NHP):
                h0 = 2 * hp
                kt_t = kt_pool.tile([128, S], F16, name="kt")
                nc.sync.dma_start(kt_t[:], ap["kT"][hp])
                v_ts = []
                for j in range(2):
                    v_t = v_pool.tile([128, NCHUNK * 65], F16, name=f"v{j}", tag=f"v{j}")
                    nc.sync.dma_start(v_t[:], ap["vaug"][h0 + j])
                    v_ts.append(v_t)
                qd_ts = []
                for j in range(2):
                    qd_t = qd_pool.tile([128, S], F16, name=f"qd{j}", tag=f"qd{j}")
                    nc.sync.dma_start(qd_t[:], ap["qTd"][h0 + j])
                    qd_ts.append(qd_t)

                # ---- phi-K + elu finish + KV accumulate ----
                kv_ps = ps_kv.tile([65, 512], F32, name="kvps", tag="kvps")
                for ch in range(NCHUNK):
                    kf_ps = ps_bigK.tile([128, 512], F32, name="kfps", tag="bigK")
                    nc.tensor.matmul(
                        kf_ps[:],
                        kt_t[:, 128 * ch : 128 * ch + 128],
                        wk_t[:],
                        start=True,
                        stop=True,
                    )
                    ek = etK_pool.tile([128, 512], F16, name="ek", tag="ek")
                    nc.scalar.activation(ek[:], kf_ps[:], AFT.Exp)
                    akk = etK_pool.tile([128, 512], F16, name="akk", tag="akk")
                    nc.vector.tensor_scalar(akk[:], ek[:], 1.0, None, ALU.min)
                    kf = kf_pool.tile([128, 512], F16, name="kf")
                    nc.vector.scalar_tensor_tensor(
                        kf[:], kf_ps[:], 0.0, akk[:], ALU.max, ALU.add
                    )
                    for j in range(2):
                        nc.tensor.matmul(
                            kv_ps[:, 256 * j : 256 * j + 256],
                            v_ts[j][:, 65 * ch : 65 * ch + 65],
                            kf[:, 256 * j : 256 * j + 256],
                            start=(ch == 0 and j == 0),
                            stop=(ch == NCHUNK - 1),
                        )

                # ---- kv -> fp16 with 1/256 fold, transpose into kstk stacks ----
                kvc_t = kvc_pool.tile([65, 512], F16, name="kvc", tag="kvc")
                nc.vector.tensor_scalar(kvc_t[:], kv_ps[:], 1.0 / OSCALE, None, ALU.mult)
                tpk_ps = ps_tpk.tile([128, 260], F32, name="tpk", tag="tpk")
                for j in range(2):
                    for c in range(C):
                        nc.tensor.matmul(
                            tpk_ps[64 * (c % 2) : 64 * (c % 2) + 64,
                                   130 * j + 65 * (c // 2) : 130 * j + 65 * (c // 2) + 65],
                            kvc_t[:, 256 * j + 64 * c : 256 * j + 64 * c + 64],
                            ident[:65, :65],
                            start=(j == 0 and c <= 1),
                            stop=True,
                        )
                kst = kstk_pool.tile([128, 260], F16, name="kstk", tag="kstk")
                nc.vector.tensor_scalar(kst[:], tpk_ps[:], 1.0, None, ALU.mult)
                kstk = {}
                for j in range(2):
                    for p in range(2):
                        kstk[(j, p)] = kst[:, 130 * j + 65 * p : 130 * j + 65 * p + 65]

                # ---- dlw: [128, 8] per stack, cols 2i+c = ksum*wmask, rest 0 ----
                dlw = {}
                for j in range(2):
                    for p in range(2):
                        i = 2 * j + p
                        dl = dl_pool.tile([128, 8], F16, name=f"dl{j}{p}", tag=f"dl{j}{p}")
                        nc.vector.tensor_scalar(dl[:], ident[:, 0:8], 0.0, None, ALU.mult)
                        ks = kstk[(j, p)][:, 64:65]
                        nc.vector.tensor_scalar(
                            dl[:, 2 * i : 2 * i + 1], ks,
                            wmask_t[:, 3 * p : 3 * p + 1], None, ALU.mult,
                        )
                        nc.vector.tensor_scalar(
                            dl[:, 2 * i + 1 : 2 * i + 2], ks,
                            wmask_t[:, 3 * p + 1 : 3 * p + 2], None, ALU.mult,
                        )
                        dlw[(j, p)] = dl

                # ---- phi-Q (c-pair stacked) + elu finish ----
                qf2 = {}
                for j in range(2):
                    for p in range(2):
                        qf_t = qf_pool.tile([128, S], F16, name=f"qf{j}{p}", tag="qf")
                        for half in range(2):
                            pq_ps = ps_bigQ.tile([128, 1024], F32, name="pqps", tag="bigQ")
                            for g in range(2):
                                o = 1024 * half + 512 * g
                                nc.tensor.matmul(
                                    pq_ps[:, 512 * g : 512 * g + 512],
                                    wq_t[p][:],
                                    qd_ts[j][:, o : o + 512],
                                    start=True,
                                    stop=True,
                                )
                            eq = etQ_pool.tile([128, 1024], F16, name="eq", tag="eq")
                            nc.scalar.activation(eq[:], pq_ps[:], AFT.Exp)
                            aq = etQ_pool.tile([128, 1024], F16, name="aq", tag="aq")
                            nc.vector.tensor_scalar(aq[:], eq[:], 1.0, None, ALU.min)
                            xr = etQ_pool.tile([128, 1024], F16, name="xr", tag="xr")
                            nc.scalar.activation(xr[:], pq_ps[:], AFT.Relu)
                            nc.vector.tensor_tensor(
                                qf_t[:, 1024 * half : 1024 * half + 1024],
                                aq[:],
                                xr[:],
                                ALU.add,
                            )
                        qf2[(j, p)] = qf_t

                # ---- Den + 1/x -> z fp16 [8, 2048] ----
                z_t = z_pool.tile([8, S], F16, name="zt", tag="z")
                for ch4 in range(4):
                    sl = slice(512 * ch4, 512 * ch4 + 512)
                    den_ps = ps_kv.tile([8, 512], F32, name="denps", tag="kvps")
                    for i, (j, p) in enumerate(((0, 0), (0, 1), (1, 0), (1, 1))):
                        nc.tensor.matmul(
                            den_ps[:],
                            dlw[(j, p)][:],
                            qf2[(j, p)][:, sl],
                            start=(i == 0),
                            stop=(i == 3),
                        )
                    lnt = lnt_pool.tile([8, 512], F32, name="lnt", tag="lnt")
                    nc.scalar.activation(lnt[:], den_ps[:], AFT.Ln)
                    nc.scalar.activation(z_t[:, sl], lnt[:], AFT.Exp, scale=-1.0)

                # ---- zrep selector matmuls, zq, OutT ----
                for ch4 in range(4):
                    sl = slice(512 * ch4, 512 * ch4 + 512)
                    ot_ps = ps_out.tile([128, 512], F32, name="otps", tag="ot")
                    for j in range(2):
                        for p in range(2):
                            i = 2 * j + p
                            zr_ps = ps_tpk.tile([128, 512], F32, name="zrps", tag="tpk")
                            nc.tensor.matmul(
                                zr_ps[:],
                                sel_t[:, 128 * i : 128 * i + 128],
                                z_t[:, sl],
                                start=True,
                                stop=True,
                            )
                            zq_t = zq_pool.tile([128, 512], F16, name="zqt", tag="zq")
                            nc.vector.tensor_tensor(
                                zq_t[:], qf2[(j, p)][:, sl], zr_ps[:], ALU.mult
                            )
                            nc.tensor.matmul(
                                ot_ps[64 * j : 64 * j + 64, :],
                                kstk[(j, p)][:, 0:64],
                                zq_t[:],
                                start=(p == 0),
                                stop=(p == 1),
                            )
                    ob = ob_pool.tile([128, 512], F16, name="ob", tag="ob")
                    nc.scalar.activation(ob[:], ot_ps[:], AFT.Copy)
                    nc.sync.dma_start(ap["outT"][h0 * 64 : h0 * 64 + 128, sl], ob[:])

    _split_multiwait(nc)
    return nc


_NC_CACHE = None


def _get_nc():
    global _NC_CACHE
    if _NC_CACHE is None:
        _NC_CACHE = build_program()
    return _NC_CACHE


def _softmax(x):
    e = np.exp(x - x.max())
    return e / e.sum()


def prep_core_inputs(queries, keys, values, key_mask, feat_W, mix_weights, core):
    n, hh = core // 2, (core % 2) * HL
    W = _softmax(np.asarray(mix_weights, np.float64)).astype(np.float32)

    qs = queries[n][:, hh : hh + HL, :].transpose(1, 2, 0)  # [HL, D, S]
    qTd = np.ascontiguousarray(
        np.concatenate([qs, qs], axis=1)
    ).astype(np.float16)  # [HL, 128, S]
    ks = keys[n][:, hh : hh + HL, :].transpose(1, 2, 0)
    kT = np.ascontiguousarray(ks.reshape(NHP, 128, S)).astype(np.float16)

    mask = key_mask[n].astype(np.float32)
    vm = values[n][:, hh : hh + HL, :] * mask[:, None, None]
    vaug = np.concatenate(
        [vm, np.broadcast_to(mask[:, None, None], (S, HL, 1))], axis=2
    )
    vaug = vaug.transpose(1, 0, 2).reshape(HL, NCHUNK, 128, 65)
    vaug = np.ascontiguousarray(vaug.transpose(0, 2, 1, 3)).reshape(
        HL, 128, NCHUNK * 65
    ).astype(np.float16)

    wq2 = np.zeros((2, 128, 128), np.float16)
    for p in range(2):
        wq2[p, :64, :64] = feat_W[2 * p]
        wq2[p, 64:, 64:] = feat_W[2 * p + 1]
    wcat = np.concatenate([feat_W[c] for c in range(C)], axis=1)
    wk = np.zeros((128, 512), np.float16)
    wk[:64, :256] = wcat
    wk[64:, 256:] = wcat

    # den lhsT scale: ksum arrives as ksum/256 (kv 1/256 fold); wmask = 1/W
    # makes den_psum = den_true/(256 W) so z = exp(-ln(den)) = 256 W / den.
    wmask = np.zeros((128, 6), np.float32)
    for p in range(2):
        wmask[:64, 3 * p + 0] = 1.0 / W[2 * p]
        wmask[64:, 3 * p + 1] = 1.0 / W[2 * p + 1]

    sel = np.zeros((8, 512), np.float16)
    for i in range(4):
        for c in range(2):
            sel[2 * i + c, 128 * i + 64 * c : 128 * i + 64 * c + 64] = 1.0

    return {"qTd": qTd, "kT": kT, "vaug": vaug, "wq2": wq2, "wk": wk,
            "wmask": wmask, "sel": sel}


def run_cores(inputs, trace=False, tmpdir=None):
    from concourse.bass_utils import run_bass_kernel_spmd

    nc = _get_nc()
    in_maps = [prep_core_inputs(**inputs, core=i) for i in range(8)]
    kwargs = {}
    if trace:
        kwargs = {"trace": True, "tmpdir": tmpdir}
    res = run_bass_kernel_spmd(nc, in_maps, core_ids=list(range(8)), **kwargs)
    out = np.empty((N, S, H, M), np.float32)
    for i in range(8):
        n, hh = i // 2, (i % 2) * HL
        oT = res.results[i]["outT"].astype(np.float32).reshape(HL, 64, S)
        for h in range(HL):
            out[n, :, hh + h, :] = oT[h].T
    return out, res


def kernel(queries, keys, values, key_mask, feat_W, mix_weights):
    out, _ = run_cores(
        dict(queries=np.asarray(queries), keys=np.asarray(keys),
             values=np.asarray(values), key_mask=np.asarray(key_mask),
             feat_W=np.asarray(feat_W), mix_weights=np.asarray(mix_weights))
    )
    return out
